# revision 15
# baseline (speedup 1.0000x reference)
"""fp8-packed attention-pooling kernel.

x ships as packed fp8 pairs (bf16 carrier = bytes (fp8(32*x[n,d]), fp8(32*x[n,d+128]))),
halving HBM traffic vs bf16. Score path: packed 16-bit PE transposes (half
cost), DoubleRow fp8 h-matmul with a two-term (hi+lo) fp8 W1 split, tanh on
ACT, tiny per-tile score matmuls, pair-batched exp on ACT. Weighted-sum
path: per-tile window matmuls (out [d, S] columns, ap_size=S) into rotating
PSUM, flushed to SBUF every FLUSH chunks; the host does the final
window->segment reduction, normalization, d-unpermutation, and adds the
exact fp8 correction term (sum(x) - sum(x8))/c^2 computed host-side.

All small constants ride in one blob DMA to avoid serialized dispatch.

Engine staggering per iteration i gives every cross-engine edge >=1
iteration of slack (LAGs: h=2 s=5 exp=9(quads) sw=10 o=11); tanh is
pair-batched and exp quad-batched to amortize ACT access-latency init:
  PE:  tp(i) | h(i-2) | s(i-5) | o(i-11)
  ACT: exp-quad(i-9) | tanh-pair(i-2..i-3)
  DVE: copy(i) | tm/sw(i-10) | flush
"""
import sys

if "/opt/trn_rl_repo" not in sys.path:
    sys.path.insert(0, "/opt/trn_rl_repo")

import ml_dtypes
import numpy as np

import concourse.bacc as bacc
import concourse.tile as tile
from concourse import bass_utils, mybir
from concourse.alu_op_type import AluOpType

C = 8
G = 1024
SPC = G // C
D = 256
H = 128
CHUNK = 1024
TPC = CHUNK // 128
NSLOT = 16
FLUSH = 8
XS = 32.0  # fp8 scale for x
WS = 64.0  # fp8 scale for W1

F32 = mybir.dt.float32
BF16 = mybir.dt.bfloat16
F8 = mybir.dt.float8e4

NPF8 = ml_dtypes.float8_e4m3
NPBF = ml_dtypes.bfloat16

_cache: dict = {}
_cache_s: dict = {}


def _blob_layout(ntiles: int, S: int):
    """bf16-column offsets of each constant inside the blob."""
    off = {}
    o = 0
    for name, width in (
        ("w1hi", 128),
        ("w1lo", 128),
        ("ident", 128),
        ("delta", ntiles),
        ("b1", 2),
        ("w2", 1),
        ("kvec", S),
        ("ones8", 1),
    ):
        off[name] = (o, o + width)
        o += width
    return off, o


def _build(npad: int, S: int):
    nchunks = npad // CHUNK
    ntiles = npad // 128
    pcols = 3 * TPC * S  # per-chunk output cols: (xhalf0, xhalf1, De) x TPC x S
    boff, CB = _blob_layout(ntiles, S)
    nc = bacc.Bacc("TRN2", target_bir_lowering=False, debug=False, num_devices=C)

    xp_d = nc.dram_tensor("xp", [128, ntiles * 128], BF16, kind="ExternalInput")
    cb_d = nc.dram_tensor("cblob", [128, CB], BF16, kind="ExternalInput")
    o_d = nc.dram_tensor("o", [128, nchunks * pcols], F32, kind="ExternalOutput")

    TANH = mybir.ActivationFunctionType.Tanh
    EXP = mybir.ActivationFunctionType.Exp
    DR = mybir.MatmulPerfMode.DoubleRow

    def bsl(ap, name):
        a, b = boff[name]
        return ap[:, a:b]

    with tile.TileContext(nc) as tc:
        with (
            tc.tile_pool(name="const", bufs=1) as constp,
            tc.tile_pool(name="xT", bufs=3) as xTp,
            tc.tile_pool(name="th", bufs=3) as thp,
            tc.tile_pool(name="eb", bufs=3) as ebp,
            tc.tile_pool(name="sw", bufs=4) as swp,
            tc.tile_pool(name="psb", bufs=1) as psbp,
            tc.tile_pool(name="ptp", bufs=2, space="PSUM") as ptpp,
            tc.tile_pool(name="ph", bufs=1, space="PSUM") as php,
            tc.tile_pool(name="ps", bufs=1, space="PSUM") as psp,
            tc.tile_pool(name="pP", bufs=1, space="PSUM") as pPp,
        ):
            blob = constp.tile([128, CB], BF16)
            slots = []
            for s_ in range(NSLOT):
                sl = constp.tile([128, TPC, 128], BF16, tag=f"slot{s_}")
                slots.append(sl)
            nc.sync.dma_start(
                slots[0][:],
                xp_d[:, 0:CHUNK].rearrange("p (j n) -> p j n", j=TPC),
            )
            nc.sync.dma_start(blob[:], cb_d[:])
            for t0_ in (1, 2):
                nc.scalar.dma_start(
                    slots[t0_][:],
                    xp_d[:, t0_ * CHUNK : (t0_ + 1) * CHUNK].rearrange(
                        "p (j n) -> p j n", j=TPC
                    ),
                )

            w1hi = bsl(blob, "w1hi").bitcast(F8).rearrange("p (i h) -> p i h", i=2)
            w1lo = bsl(blob, "w1lo").bitcast(F8).rearrange("p (i h) -> p i h", i=2)
            ident = bsl(blob, "ident")
            delta = bsl(blob, "delta")
            b1 = bsl(blob, "b1").bitcast(F32)
            w2 = bsl(blob, "w2")
            kvec = bsl(blob, "kvec")
            ones8 = bsl(blob, "ones8").bitcast(F8)[:, 0:1]

            # staging for P flush groups (written by DVE, DMA'd out per group)
            psb = psbp.tile([128, nchunks * pcols], F32)

            xT_tiles = {}
            th_tiles = {}
            ps_tiles = {}
            eb_tiles = {}
            pP_tiles = {}
            sw_tiles = {}

            def nat_dma(t):
                nc.sync.dma_start(
                    slots[t % NSLOT][:],
                    xp_d[:, t * CHUNK : (t + 1) * CHUNK].rearrange(
                        "p (j n) -> p j n", j=TPC
                    ),
                )

            def stage_tp(t):
                xn = slots[t % NSLOT]
                ptp = ptpp.tile([128, TPC, 128], BF16)
                for j in range(TPC):
                    nc.tensor.transpose(ptp[:, j, :], xn[:, j, :], ident)
                xT = xTp.tile([128, TPC, 128], BF16)
                xT_tiles[t] = xT
                nc.vector.tensor_copy(xT[:], ptp[:])

            ph_tiles = {}

            def stage_h(t):
                xT = xT_tiles.pop(t)
                # fp8 view: [p, i, (j n)] where i selects the byte (d-half)
                rhs = xT[:].bitcast(F8).rearrange("p j (n i) -> p i (j n)", i=2)
                if t % 2 == 0:
                    ph_tiles[t] = php.tile([H, 2, CHUNK], F32, name="ph", tag="ph")
                ph = ph_tiles[t - t % 2]
                for u in range(CHUNK // 512):
                    rv = rhs[:, :, u * 512 : (u + 1) * 512]
                    nc.tensor.matmul(
                        ph[:, t % 2, u * 512 : (u + 1) * 512], w1hi, rv,
                        start=True, stop=False, perf_mode=DR,
                    )
                    nc.tensor.matmul(
                        ph[:, t % 2, u * 512 : (u + 1) * 512], w1lo, rv,
                        start=False, stop=True, perf_mode=DR,
                    )
                if t % 2 == 1 or t == nchunks - 1:
                    a = t - t % 2
                    w = t % 2 + 1
                    ph = ph_tiles.pop(a)
                    th = thp.tile([H, 2, CHUNK], BF16, name="th", tag="th")
                    th_tiles[a] = th
                    nc.scalar.activation(
                        th[:, 0:w], ph[:, 0:w], TANH, bias=b1,
                        scale=1.0 / (XS * WS),
                    )

            def stage_s(t):
                th = th_tiles[t - t % 2]
                if t % 2 == 1 or t == nchunks - 1:
                    del th_tiles[t - t % 2]
                if t % 4 == 0:
                    ps_tiles[t] = psp.tile([128, 4, TPC], F32, name="ps", tag="ps")
                ps = ps_tiles[t - t % 4]
                for j in range(TPC):
                    nc.tensor.matmul(
                        ps[:, t % 4, j : j + 1],
                        th[:, t % 2, j * 128 : (j + 1) * 128],
                        w2,
                        start=True,
                        stop=True,
                    )

            def stage_exp(a):
                # quad (a..a+3); a % 4 == 0. Covers up to four chunks.
                w = min(4, nchunks - a)
                ps = ps_tiles.pop(a)
                eb = ebp.tile([128, 4, TPC], BF16)
                eb_tiles[a] = eb
                nc.scalar.activation(
                    eb[:, 0:w], ps[:, 0:w], EXP, bias=0.0, scale=1.0
                )

            def stage_sw(t):
                eb = eb_tiles[t - t % 4]
                if t % 4 == 3 or t == nchunks - 1:
                    del eb_tiles[t - t % 4]
                tm = swp.tile([128, TPC, S], BF16, tag="tm")
                nc.vector.tensor_tensor(
                    tm[:],
                    delta[:, t * TPC : (t + 1) * TPC].unsqueeze(2).broadcast_to(
                        [128, TPC, S]
                    ),
                    kvec.unsqueeze(1).broadcast_to([128, TPC, S]),
                    AluOpType.is_equal,
                )
                sw = swp.tile([128, TPC, S], BF16, tag="sw")
                nc.vector.tensor_tensor(
                    sw[:],
                    tm[:],
                    eb[:, t % 4].unsqueeze(2).broadcast_to([128, TPC, S]),
                    AluOpType.mult,
                )
                return sw

            def stage_o(t, sw):
                g, fi = divmod(t, FLUSH)
                if fi == 0:
                    pP_tiles[g] = pPp.tile(
                        [128, FLUSH, 3, TPC, S], F32, name="pP", tag="pP"
                    )
                pP = pP_tiles[g]
                xn8 = slots[t % NSLOT][:].bitcast(F8)  # [128, TPC, 256]
                for j in range(TPC):
                    nc.tensor.matmul(
                        pP[:, fi, 0, j, :], xn8[:, j, 0:128], sw[:, j, :],
                        start=True, stop=True,
                    )
                    nc.tensor.matmul(
                        pP[:, fi, 1, j, :], xn8[:, j, 128:256], sw[:, j, :],
                        start=True, stop=True,
                    )
                    nc.tensor.matmul(
                        pP[0:1, fi, 2, j, :], ones8, sw[:, j, :],
                        start=True, stop=True,
                    )
                # flush the group once its last chunk is done
                if fi == FLUSH - 1 or t == nchunks - 1:
                    pP = pP_tiles.pop(g)
                    nw = fi + 1
                    nc.vector.tensor_copy(
                        psb[:, g * FLUSH * pcols : (g * FLUSH + nw) * pcols]
                        .rearrange("p (f c) -> p f c", f=nw),
                        pP[:, 0:nw].rearrange("p f h j k -> p f (h j k)"),
                    )
                    nc.sync.dma_start(
                        o_d[:, g * FLUSH * pcols : (g * FLUSH + nw) * pcols],
                        psb[:, g * FLUSH * pcols : (g * FLUSH + nw) * pcols],
                    )

            LAG_H, LAG_S, LAG_E, LAG_W, LAG_O = 2, 5, 9, 10, 11
            for t in range(nchunks + LAG_O):
                if t + 3 < nchunks:
                    nat_dma(t + 3)
                a = t - LAG_E
                if 0 <= a < nchunks and a % 4 == 0:
                    stage_exp(a)
                if t < nchunks:
                    stage_tp(t)
                if 0 <= t - LAG_H < nchunks:
                    stage_h(t - LAG_H)
                if 0 <= t - LAG_S < nchunks:
                    stage_s(t - LAG_S)
                if 0 <= t - LAG_W < nchunks:
                    sw_tiles[t - LAG_W] = stage_sw(t - LAG_W)
                if 0 <= t - LAG_O < nchunks:
                    stage_o(t - LAG_O, sw_tiles.pop(t - LAG_O))

    nc.compile()
    return nc


def kernel(x, batch, W1, b1, W2, b2):
    x = np.asarray(x, np.float32)
    batch = np.asarray(batch)
    W1 = np.asarray(W1, np.float32)
    b1 = np.asarray(b1, np.float32)
    W2 = np.asarray(W2, np.float32)

    bat = batch.astype(np.int64)
    N = bat.shape[0]
    bounds = np.searchsorted(bat, np.arange(0, G + 1, SPC), side="left")
    ncounts = np.diff(bounds)
    npad = int(-(-ncounts.max() // CHUNK) * CHUNK)
    ntiles = npad // 128
    nchunks = npad // CHUNK

    counts = np.bincount(bat, minlength=G).astype(np.float32)

    # global fp8 quantization (scaled), plus exact residual for the host-side
    # correction term
    x8 = (x * XS).astype(NPF8)
    x8f = x8.astype(np.float32)
    resid = x - x8f * (1.0 / XS)  # exact in f32

    # per-segment sums of the residual -> correction (sum x - sum x8)/c^2
    seg_starts = np.searchsorted(bat, np.arange(G), side="left")
    rsum = np.add.reduceat(resid, np.minimum(seg_starts, N - 1), axis=0)
    # reduceat yields a[i] (not 0) for empty segments; zero those out
    rsum[counts == 0] = 0.0

    cg = np.maximum(counts, 1.0)
    ccorr = rsum / (cg * cg)[:, None]  # [G, D]

    # W1 two-term fp8 split (scaled by WS); DoubleRow pack [c, i, h]
    w1s = W1 * WS
    w1hi8 = w1s.astype(NPF8)
    w1lo8 = (w1s - w1hi8.astype(np.float32)).astype(NPF8)
    w1hi = np.ascontiguousarray(
        np.stack([w1hi8[:128], w1hi8[128:]], axis=1)
    )  # [128, 2, H] fp8
    w1lo = np.ascontiguousarray(np.stack([w1lo8[:128], w1lo8[128:]], axis=1))

    # per-core prep
    in_maps = []
    metas = []
    S = 2
    core_data = []
    for c in range(C):
        s, e = bounds[c], bounds[c + 1]
        nct = e - s
        locseg = (bat[s:e] - c * SPC).astype(np.int64)
        g0 = np.zeros(ntiles, np.int64)
        nvalid_tiles = -(-nct // 128)
        if nct:
            g0[:nvalid_tiles] = locseg[np.arange(nvalid_tiles) * 128]
        dlt = np.full(npad, -1.0, np.float32)
        if nct:
            dlt[:nct] = locseg - g0[np.arange(nct) // 128]
        smax = int(dlt.max()) + 1 if nct else 1
        core_data.append((s, e, nct, g0, dlt))
        S = max(S, smax)

    key = (npad, S)
    if key not in _cache_s:
        _cache_s[key] = _build(npad, S)
    nc = _cache_s[key]
    _cache[npad] = nc  # test.py compatibility

    pcols = 3 * TPC * S
    boff, CB = _blob_layout(ntiles, S)

    for c in range(C):
        s, e, nct, g0, dlt = core_data[c]
        xpad = np.zeros((npad, D), NPF8)
        xpad[:nct] = x8[s:e]
        xb = xpad.view(np.uint8)
        pk = (
            xb[:, :128].astype(np.uint16)
            | (xb[:, 128:].astype(np.uint16) << 8)
        )  # [npad, 128] uint16
        xp = np.ascontiguousarray(
            pk.reshape(ntiles, 128, 128).transpose(1, 0, 2).reshape(128, ntiles * 128)
        ).view(NPBF)

        # constant blob, byte-assembled then viewed as bf16 columns
        bb = np.zeros((128, CB * 2), np.uint8)

        def put(name, arr_bytes):
            a, b = boff[name]
            bb[:, a * 2 : a * 2 + arr_bytes.shape[1]] = arr_bytes

        put("w1hi", w1hi.reshape(128, 256).view(np.uint8))
        put("w1lo", w1lo.reshape(128, 256).view(np.uint8))
        put("ident", np.eye(128, dtype=NPBF).view(np.uint8))
        dl = np.ascontiguousarray(dlt.reshape(ntiles, 128).T.astype(NPBF))
        put("delta", dl.view(np.uint8))
        put("b1", b1.reshape(H, 1).astype(np.float32).view(np.uint8))
        put("w2", W2.reshape(H, 1).astype(NPBF).view(np.uint8))
        kv = np.broadcast_to(
            np.arange(S, dtype=np.float32)[None, :], (128, S)
        ).astype(NPBF)
        put("kvec", np.ascontiguousarray(kv).view(np.uint8))
        put("ones8", np.ones((128, 1), NPF8).view(np.uint8))

        in_maps.append({"xp": xp, "cblob": bb.view(NPBF)})
        metas.append((g0, nct))

    res = bass_utils.run_bass_kernel_spmd(nc, in_maps, core_ids=list(range(C)))

    # host-side finish: window->segment reduction, normalize, unpermute, correct
    f_idx = np.arange(256)
    d_of_f = f_idx // 2 + 128 * (f_idx % 2)
    f_of_d = np.empty(256, np.int64)
    f_of_d[d_of_f] = f_idx

    out = np.zeros((G, D), np.float32)
    for c in range(C):
        g0, nct = metas[c]
        o = res.results[c]["o"].reshape(128, nchunks, 3, TPC, S)
        # P rows: [q, t, half, j, k]; f = half*128 + q
        P = np.concatenate([o[:, :, 0], o[:, :, 1]], axis=0)  # [256, t, j, k]
        De = o[0, :, 2]  # [t, j, k]
        P = P.reshape(256, ntiles, S)
        De = De.reshape(ntiles, S)
        wseg = np.minimum(g0[:, None] + np.arange(S)[None, :], SPC - 1)
        U = np.zeros((SPC, 256), np.float64)
        np.add.at(U, wseg.ravel(), P.reshape(256, -1).T.astype(np.float64))
        DeU = np.zeros(SPC, np.float64)
        np.add.at(DeU, wseg.ravel(), De.ravel().astype(np.float64))
        cgl = cg[c * SPC : (c + 1) * SPC]
        y = U[:, f_of_d] / (XS * np.maximum(DeU, 1e-30) * cgl)[:, None]
        out[c * SPC : (c + 1) * SPC] = y.astype(np.float32) + ccorr[
            c * SPC : (c + 1) * SPC
        ]
    return out


# revision 16
# speedup vs baseline: 1.2418x; 1.2418x over previous
"""fp8-packed attention-pooling kernel.

x ships as packed fp8 pairs (bf16 carrier = bytes (fp8(32*x[n,d]), fp8(32*x[n,d+128]))),
halving HBM traffic vs bf16. Score path: packed 16-bit PE transposes (half
cost), DoubleRow fp8 h-matmul with a two-term (hi+lo) fp8 W1 split, tanh on
ACT, tiny per-tile score matmuls, pair-batched exp on ACT. Weighted-sum
path: per-tile window matmuls (out [d, S] columns, ap_size=S) into rotating
PSUM, flushed to SBUF every FLUSH chunks; the host does the final
window->segment reduction, normalization, d-unpermutation, and adds the
exact fp8 correction term (sum(x) - sum(x8))/c^2 computed host-side.

All small constants ride in one blob DMA to avoid serialized dispatch.

Engine staggering per iteration i gives every cross-engine edge >=1
iteration of slack (LAGs: h=2 s=5 exp=9(quads) sw=10 o=11); tanh is
pair-batched and exp quad-batched to amortize ACT access-latency init:
  PE:  tp(i) | h(i-2) | s(i-5) | o(i-11)
  ACT: exp-quad(i-9) | tanh-pair(i-2..i-3)
  DVE: copy(i) | tm/sw(i-10) | flush
"""
import sys

if "/opt/trn_rl_repo" not in sys.path:
    sys.path.insert(0, "/opt/trn_rl_repo")

import ml_dtypes
import numpy as np

import concourse.bacc as bacc
import concourse.tile as tile
from concourse import bass_utils, mybir
from concourse.alu_op_type import AluOpType

C = 8
G = 1024
SPC = G // C
D = 256
H = 128
CHUNK = 1024
TPC = CHUNK // 128
NSLOT = 16
FLUSH = 8
XS = 32.0  # fp8 scale for x
WS = 64.0  # fp8 scale for W1

F32 = mybir.dt.float32
BF16 = mybir.dt.bfloat16
F8 = mybir.dt.float8e4

NPF8 = ml_dtypes.float8_e4m3
NPBF = ml_dtypes.bfloat16

_cache: dict = {}
_cache_s: dict = {}


def _blob_layout(ntiles: int, S: int):
    """bf16-column offsets of each constant inside the blob."""
    off = {}
    o = 0
    for name, width in (
        ("w1hi", 128),
        ("w1lo", 128),
        ("ident", 128),
        ("delta", ntiles),
        ("b1", 2),
        ("w2", 1),
        ("kvec", S),
        ("ones8", 1),
    ):
        off[name] = (o, o + width)
        o += width
    return off, o


def _build(npad: int, S: int):
    nchunks = npad // CHUNK
    ntiles = npad // 128
    pcols = 3 * TPC * S  # per-chunk output cols: (xhalf0, xhalf1, De) x TPC x S
    boff, CB = _blob_layout(ntiles, S)
    nc = bacc.Bacc("TRN2", target_bir_lowering=False, debug=False, num_devices=C)

    xp_d = nc.dram_tensor("xp", [128, ntiles * 128], BF16, kind="ExternalInput")
    cb_d = nc.dram_tensor("cblob", [128, CB], BF16, kind="ExternalInput")
    o_d = nc.dram_tensor("o", [128, nchunks * pcols], F32, kind="ExternalOutput")

    TANH = mybir.ActivationFunctionType.Tanh
    EXP = mybir.ActivationFunctionType.Exp
    DR = mybir.MatmulPerfMode.DoubleRow

    def bsl(ap, name):
        a, b = boff[name]
        return ap[:, a:b]

    with tile.TileContext(nc) as tc:
        with (
            tc.tile_pool(name="const", bufs=1) as constp,
            tc.tile_pool(name="xT", bufs=3) as xTp,
            tc.tile_pool(name="th", bufs=3) as thp,
            tc.tile_pool(name="eb", bufs=3) as ebp,
            tc.tile_pool(name="sw", bufs=4) as swp,
            tc.tile_pool(name="psb", bufs=1) as psbp,
            tc.tile_pool(name="ptp", bufs=2, space="PSUM") as ptpp,
            tc.tile_pool(name="ph", bufs=2, space="PSUM") as php,
            tc.tile_pool(name="ps", bufs=1, space="PSUM") as psp,
            tc.tile_pool(name="pP", bufs=1, space="PSUM") as pPp,
        ):
            blob = constp.tile([128, CB], BF16)
            slots = []
            for s_ in range(NSLOT):
                sl = constp.tile([128, TPC, 128], BF16, tag=f"slot{s_}")
                slots.append(sl)
            nc.sync.dma_start(
                slots[0][:],
                xp_d[:, 0:CHUNK].rearrange("p (j n) -> p j n", j=TPC),
            )
            nc.sync.dma_start(blob[:], cb_d[:])
            for t0_ in (1, 2):
                nc.scalar.dma_start(
                    slots[t0_][:],
                    xp_d[:, t0_ * CHUNK : (t0_ + 1) * CHUNK].rearrange(
                        "p (j n) -> p j n", j=TPC
                    ),
                )

            w1hi = bsl(blob, "w1hi").bitcast(F8).rearrange("p (i h) -> p i h", i=2)
            w1lo = bsl(blob, "w1lo").bitcast(F8).rearrange("p (i h) -> p i h", i=2)
            ident = bsl(blob, "ident")
            delta = bsl(blob, "delta")
            b1 = bsl(blob, "b1").bitcast(F32)
            w2 = bsl(blob, "w2")
            kvec = bsl(blob, "kvec")
            ones8 = bsl(blob, "ones8").bitcast(F8)[:, 0:1]

            # staging for P flush groups (written by DVE, DMA'd out per group)
            psb = psbp.tile([128, nchunks * pcols], F32)

            xT_tiles = {}
            th_tiles = {}
            ps_tiles = {}
            eb_tiles = {}
            pP_tiles = {}
            sw_tiles = {}

            def nat_dma(t):
                nc.sync.dma_start(
                    slots[t % NSLOT][:],
                    xp_d[:, t * CHUNK : (t + 1) * CHUNK].rearrange(
                        "p (j n) -> p j n", j=TPC
                    ),
                )

            def stage_tp(t):
                xn = slots[t % NSLOT]
                ptp = ptpp.tile([128, TPC, 128], BF16)
                for j in range(TPC):
                    nc.tensor.transpose(ptp[:, j, :], xn[:, j, :], ident)
                xT = xTp.tile([128, TPC, 128], BF16)
                xT_tiles[t] = xT
                nc.vector.tensor_copy(xT[:], ptp[:])

            def stage_h(t):
                xT = xT_tiles.pop(t)
                # fp8 view: [p, i, (j n)] where i selects the byte (d-half)
                rhs = xT[:].bitcast(F8).rearrange("p j (n i) -> p i (j n)", i=2)
                ph = php.tile([H, CHUNK], F32)
                for u in range(CHUNK // 512):
                    rv = rhs[:, :, u * 512 : (u + 1) * 512]
                    nc.tensor.matmul(
                        ph[:, u * 512 : (u + 1) * 512], w1hi, rv,
                        start=True, stop=False, perf_mode=DR,
                    )
                    nc.tensor.matmul(
                        ph[:, u * 512 : (u + 1) * 512], w1lo, rv,
                        start=False, stop=True, perf_mode=DR,
                    )
                th = thp.tile([H, CHUNK], BF16)
                th_tiles[t] = th
                nc.scalar.activation(
                    th[:], ph[:], TANH, bias=b1, scale=1.0 / (XS * WS)
                )

            def stage_s(t):
                th = th_tiles.pop(t)
                if t % 4 == 0:
                    ps_tiles[t] = psp.tile([128, 4, TPC], F32, name="ps", tag="ps")
                ps = ps_tiles[t - t % 4]
                for j in range(TPC):
                    nc.tensor.matmul(
                        ps[:, t % 4, j : j + 1],
                        th[:, j * 128 : (j + 1) * 128],
                        w2,
                        start=True,
                        stop=True,
                    )

            def stage_exp(a):
                # quad (a..a+3); a % 4 == 0. Covers up to four chunks.
                w = min(4, nchunks - a)
                ps = ps_tiles.pop(a)
                eb = ebp.tile([128, 4, TPC], BF16)
                eb_tiles[a] = eb
                nc.scalar.activation(
                    eb[:, 0:w], ps[:, 0:w], EXP, bias=0.0, scale=1.0
                )

            def stage_sw(t):
                eb = eb_tiles[t - t % 4]
                if t % 4 == 3 or t == nchunks - 1:
                    del eb_tiles[t - t % 4]
                tm = swp.tile([128, TPC, S], BF16, tag="tm")
                nc.vector.tensor_tensor(
                    tm[:],
                    delta[:, t * TPC : (t + 1) * TPC].unsqueeze(2).broadcast_to(
                        [128, TPC, S]
                    ),
                    kvec.unsqueeze(1).broadcast_to([128, TPC, S]),
                    AluOpType.is_equal,
                )
                sw = swp.tile([128, TPC, S], BF16, tag="sw")
                nc.vector.tensor_tensor(
                    sw[:],
                    tm[:],
                    eb[:, t % 4].unsqueeze(2).broadcast_to([128, TPC, S]),
                    AluOpType.mult,
                )
                return sw

            def stage_o(t, sw):
                g, fi = divmod(t, FLUSH)
                if fi == 0:
                    pP_tiles[g] = pPp.tile(
                        [128, FLUSH, 3, TPC, S], F32, name="pP", tag="pP"
                    )
                pP = pP_tiles[g]
                xn8 = slots[t % NSLOT][:].bitcast(F8)  # [128, TPC, 256]
                for j in range(TPC):
                    nc.tensor.matmul(
                        pP[:, fi, 0, j, :], xn8[:, j, 0:128], sw[:, j, :],
                        start=True, stop=True,
                    )
                    nc.tensor.matmul(
                        pP[:, fi, 1, j, :], xn8[:, j, 128:256], sw[:, j, :],
                        start=True, stop=True,
                    )
                    nc.tensor.matmul(
                        pP[0:1, fi, 2, j, :], ones8, sw[:, j, :],
                        start=True, stop=True,
                    )
                # flush the group once its last chunk is done
                if fi == FLUSH - 1 or t == nchunks - 1:
                    pP = pP_tiles.pop(g)
                    nw = fi + 1
                    nc.vector.tensor_copy(
                        psb[:, g * FLUSH * pcols : (g * FLUSH + nw) * pcols]
                        .rearrange("p (f c) -> p f c", f=nw),
                        pP[:, 0:nw].rearrange("p f h j k -> p f (h j k)"),
                    )
                    nc.sync.dma_start(
                        o_d[:, g * FLUSH * pcols : (g * FLUSH + nw) * pcols],
                        psb[:, g * FLUSH * pcols : (g * FLUSH + nw) * pcols],
                    )

            LAG_H, LAG_S, LAG_E, LAG_W, LAG_O = 2, 5, 9, 10, 11
            for t in range(nchunks + LAG_O):
                if t + 3 < nchunks:
                    nat_dma(t + 3)
                a = t - LAG_E
                if 0 <= a < nchunks and a % 4 == 0:
                    stage_exp(a)
                if t < nchunks:
                    stage_tp(t)
                if 0 <= t - LAG_H < nchunks:
                    stage_h(t - LAG_H)
                if 0 <= t - LAG_S < nchunks:
                    stage_s(t - LAG_S)
                if 0 <= t - LAG_W < nchunks:
                    sw_tiles[t - LAG_W] = stage_sw(t - LAG_W)
                if 0 <= t - LAG_O < nchunks:
                    stage_o(t - LAG_O, sw_tiles.pop(t - LAG_O))

    nc.compile()
    return nc


def kernel(x, batch, W1, b1, W2, b2):
    x = np.asarray(x, np.float32)
    batch = np.asarray(batch)
    W1 = np.asarray(W1, np.float32)
    b1 = np.asarray(b1, np.float32)
    W2 = np.asarray(W2, np.float32)

    bat = batch.astype(np.int64)
    N = bat.shape[0]
    bounds = np.searchsorted(bat, np.arange(0, G + 1, SPC), side="left")
    ncounts = np.diff(bounds)
    npad = int(-(-ncounts.max() // CHUNK) * CHUNK)
    ntiles = npad // 128
    nchunks = npad // CHUNK

    counts = np.bincount(bat, minlength=G).astype(np.float32)

    # global fp8 quantization (scaled), plus exact residual for the host-side
    # correction term
    x8 = (x * XS).astype(NPF8)
    x8f = x8.astype(np.float32)
    resid = x - x8f * (1.0 / XS)  # exact in f32

    # per-segment sums of the residual -> correction (sum x - sum x8)/c^2
    seg_starts = np.searchsorted(bat, np.arange(G), side="left")
    rsum = np.add.reduceat(resid, np.minimum(seg_starts, N - 1), axis=0)
    # reduceat yields a[i] (not 0) for empty segments; zero those out
    rsum[counts == 0] = 0.0

    cg = np.maximum(counts, 1.0)
    ccorr = rsum / (cg * cg)[:, None]  # [G, D]

    # W1 two-term fp8 split (scaled by WS); DoubleRow pack [c, i, h]
    w1s = W1 * WS
    w1hi8 = w1s.astype(NPF8)
    w1lo8 = (w1s - w1hi8.astype(np.float32)).astype(NPF8)
    w1hi = np.ascontiguousarray(
        np.stack([w1hi8[:128], w1hi8[128:]], axis=1)
    )  # [128, 2, H] fp8
    w1lo = np.ascontiguousarray(np.stack([w1lo8[:128], w1lo8[128:]], axis=1))

    # per-core prep
    in_maps = []
    metas = []
    S = 2
    core_data = []
    for c in range(C):
        s, e = bounds[c], bounds[c + 1]
        nct = e - s
        locseg = (bat[s:e] - c * SPC).astype(np.int64)
        g0 = np.zeros(ntiles, np.int64)
        nvalid_tiles = -(-nct // 128)
        if nct:
            g0[:nvalid_tiles] = locseg[np.arange(nvalid_tiles) * 128]
        dlt = np.full(npad, -1.0, np.float32)
        if nct:
            dlt[:nct] = locseg - g0[np.arange(nct) // 128]
        smax = int(dlt.max()) + 1 if nct else 1
        core_data.append((s, e, nct, g0, dlt))
        S = max(S, smax)

    key = (npad, S)
    if key not in _cache_s:
        _cache_s[key] = _build(npad, S)
    nc = _cache_s[key]
    _cache[npad] = nc  # test.py compatibility

    pcols = 3 * TPC * S
    boff, CB = _blob_layout(ntiles, S)

    for c in range(C):
        s, e, nct, g0, dlt = core_data[c]
        xpad = np.zeros((npad, D), NPF8)
        xpad[:nct] = x8[s:e]
        xb = xpad.view(np.uint8)
        pk = (
            xb[:, :128].astype(np.uint16)
            | (xb[:, 128:].astype(np.uint16) << 8)
        )  # [npad, 128] uint16
        xp = np.ascontiguousarray(
            pk.reshape(ntiles, 128, 128).transpose(1, 0, 2).reshape(128, ntiles * 128)
        ).view(NPBF)

        # constant blob, byte-assembled then viewed as bf16 columns
        bb = np.zeros((128, CB * 2), np.uint8)

        def put(name, arr_bytes):
            a, b = boff[name]
            bb[:, a * 2 : a * 2 + arr_bytes.shape[1]] = arr_bytes

        put("w1hi", w1hi.reshape(128, 256).view(np.uint8))
        put("w1lo", w1lo.reshape(128, 256).view(np.uint8))
        put("ident", np.eye(128, dtype=NPBF).view(np.uint8))
        dl = np.ascontiguousarray(dlt.reshape(ntiles, 128).T.astype(NPBF))
        put("delta", dl.view(np.uint8))
        put("b1", b1.reshape(H, 1).astype(np.float32).view(np.uint8))
        put("w2", W2.reshape(H, 1).astype(NPBF).view(np.uint8))
        kv = np.broadcast_to(
            np.arange(S, dtype=np.float32)[None, :], (128, S)
        ).astype(NPBF)
        put("kvec", np.ascontiguousarray(kv).view(np.uint8))
        put("ones8", np.ones((128, 1), NPF8).view(np.uint8))

        in_maps.append({"xp": xp, "cblob": bb.view(NPBF)})
        metas.append((g0, nct))

    res = bass_utils.run_bass_kernel_spmd(nc, in_maps, core_ids=list(range(C)))

    # host-side finish: window->segment reduction, normalize, unpermute, correct
    f_idx = np.arange(256)
    d_of_f = f_idx // 2 + 128 * (f_idx % 2)
    f_of_d = np.empty(256, np.int64)
    f_of_d[d_of_f] = f_idx

    out = np.zeros((G, D), np.float32)
    for c in range(C):
        g0, nct = metas[c]
        o = res.results[c]["o"].reshape(128, nchunks, 3, TPC, S)
        # P rows: [q, t, half, j, k]; f = half*128 + q
        P = np.concatenate([o[:, :, 0], o[:, :, 1]], axis=0)  # [256, t, j, k]
        De = o[0, :, 2]  # [t, j, k]
        P = P.reshape(256, ntiles, S)
        De = De.reshape(ntiles, S)
        wseg = np.minimum(g0[:, None] + np.arange(S)[None, :], SPC - 1)
        U = np.zeros((SPC, 256), np.float64)
        np.add.at(U, wseg.ravel(), P.reshape(256, -1).T.astype(np.float64))
        DeU = np.zeros(SPC, np.float64)
        np.add.at(DeU, wseg.ravel(), De.ravel().astype(np.float64))
        cgl = cg[c * SPC : (c + 1) * SPC]
        y = U[:, f_of_d] / (XS * np.maximum(DeU, 1e-30) * cgl)[:, None]
        out[c * SPC : (c + 1) * SPC] = y.astype(np.float32) + ccorr[
            c * SPC : (c + 1) * SPC
        ]
    return out


# revision 17
# speedup vs baseline: 1.3144x; 1.0584x over previous
"""fp8-packed attention-pooling kernel.

x ships as packed fp8 pairs (bf16 carrier = bytes (fp8(32*x[n,d]), fp8(32*x[n,d+128]))),
halving HBM traffic vs bf16. Score path: packed 16-bit PE transposes (half
cost), DoubleRow fp8 h-matmul with a two-term (hi+lo) fp8 W1 split, tanh on
ACT, tiny per-tile score matmuls, pair-batched exp on ACT. Weighted-sum
path: per-tile window matmuls (out [d, S] columns, ap_size=S) into rotating
PSUM, flushed to SBUF every FLUSH chunks; the host does the final
window->segment reduction, normalization, d-unpermutation, and adds the
exact fp8 correction term (sum(x) - sum(x8))/c^2 computed host-side.

All small constants ride in one blob DMA to avoid serialized dispatch.

Engine staggering per iteration i gives every cross-engine edge >=1
iteration of slack (LAGs: h=2 s=5 exp=9(quads) sw=10 o=11); tanh is
pair-batched and exp quad-batched to amortize ACT access-latency init:
  PE:  tp(i) | h(i-2) | s(i-5) | o(i-11)
  ACT: exp-quad(i-9) | tanh-pair(i-2..i-3)
  DVE: copy(i) | tm/sw(i-10) | flush
"""
import sys

if "/opt/trn_rl_repo" not in sys.path:
    sys.path.insert(0, "/opt/trn_rl_repo")

import ml_dtypes
import numpy as np

import concourse.bacc as bacc
import concourse.tile as tile
from concourse import bass_utils, mybir
from concourse.alu_op_type import AluOpType

C = 8
G = 1024
SPC = G // C
D = 256
H = 128
CHUNK = 1024
TPC = CHUNK // 128
NSLOT = 12
FLUSH = 8
XS = 32.0  # fp8 scale for x
WS = 64.0  # fp8 scale for W1

F32 = mybir.dt.float32
BF16 = mybir.dt.bfloat16
F8 = mybir.dt.float8e4

NPF8 = ml_dtypes.float8_e4m3
NPBF = ml_dtypes.bfloat16

_cache: dict = {}
_cache_s: dict = {}


def _blob_layout(ntiles: int, S: int):
    """bf16-column offsets of each constant inside the blob."""
    off = {}
    o = 0
    for name, width in (
        ("w1hi", 128),
        ("w1lo", 128),
        ("ident", 128),
        ("delta", ntiles),
        ("b1", 2),
        ("w2", 1),
        ("kvec", S),
        ("ones8", 1),
    ):
        off[name] = (o, o + width)
        o += width
    return off, o


def _build(npad: int, S: int):
    nchunks = npad // CHUNK
    ntiles = npad // 128
    pcols = 3 * TPC * S  # per-chunk output cols: (xhalf0, xhalf1, De) x TPC x S
    boff, CB = _blob_layout(ntiles, S)
    nc = bacc.Bacc("TRN2", target_bir_lowering=False, debug=False, num_devices=C)

    xp_d = nc.dram_tensor("xp", [128, ntiles * 128], BF16, kind="ExternalInput")
    cb_d = nc.dram_tensor("cblob", [128, CB], BF16, kind="ExternalInput")
    o_d = nc.dram_tensor("o", [128, nchunks * pcols], F32, kind="ExternalOutput")

    TANH = mybir.ActivationFunctionType.Tanh
    EXP = mybir.ActivationFunctionType.Exp
    DR = mybir.MatmulPerfMode.DoubleRow

    def bsl(ap, name):
        a, b = boff[name]
        return ap[:, a:b]

    with tile.TileContext(nc) as tc:
        with (
            tc.tile_pool(name="const", bufs=1) as constp,
            tc.tile_pool(name="xT", bufs=3) as xTp,
            tc.tile_pool(name="th", bufs=3) as thp,
            tc.tile_pool(name="eb", bufs=3) as ebp,
            tc.tile_pool(name="sw", bufs=4) as swp,
            tc.tile_pool(name="psb", bufs=1) as psbp,
            tc.tile_pool(name="ptp", bufs=2, space="PSUM") as ptpp,
            tc.tile_pool(name="ph", bufs=2, space="PSUM") as php,
            tc.tile_pool(name="ps", bufs=1, space="PSUM") as psp,
            tc.tile_pool(name="pP", bufs=1, space="PSUM") as pPp,
        ):
            blob = constp.tile([128, CB], BF16)
            slots = []
            for s_ in range(NSLOT):
                sl = constp.tile([128, TPC, 128], BF16, tag=f"slot{s_}")
                slots.append(sl)
            nc.sync.dma_start(
                slots[0][:],
                xp_d[:, 0:CHUNK].rearrange("p (j n) -> p j n", j=TPC),
            )
            nc.sync.dma_start(blob[:], cb_d[:])
            for t0_ in (1, 2):
                nc.scalar.dma_start(
                    slots[t0_][:],
                    xp_d[:, t0_ * CHUNK : (t0_ + 1) * CHUNK].rearrange(
                        "p (j n) -> p j n", j=TPC
                    ),
                )

            w1hi = bsl(blob, "w1hi").bitcast(F8).rearrange("p (i h) -> p i h", i=2)
            w1lo = bsl(blob, "w1lo").bitcast(F8).rearrange("p (i h) -> p i h", i=2)
            ident = bsl(blob, "ident")
            delta = bsl(blob, "delta")
            b1 = bsl(blob, "b1").bitcast(F32)
            w2 = bsl(blob, "w2")
            kvec = bsl(blob, "kvec")
            ones8 = bsl(blob, "ones8").bitcast(F8)[:, 0:1]

            # staging for P flush groups (written by DVE, DMA'd out per group)
            psb = psbp.tile([128, nchunks * pcols], F32)

            xT_tiles = {}
            th_tiles = {}
            ps_tiles = {}
            eb_tiles = {}
            pP_tiles = {}
            sw_tiles = {}

            def nat_dma(t):
                nc.sync.dma_start(
                    slots[t % NSLOT][:],
                    xp_d[:, t * CHUNK : (t + 1) * CHUNK].rearrange(
                        "p (j n) -> p j n", j=TPC
                    ),
                )

            def stage_tp(t):
                xn = slots[t % NSLOT]
                ptp = ptpp.tile([128, TPC, 128], BF16)
                for j in range(TPC):
                    nc.tensor.transpose(ptp[:, j, :], xn[:, j, :], ident)
                xT = xTp.tile([128, TPC, 128], BF16)
                xT_tiles[t] = xT
                nc.vector.tensor_copy(xT[:], ptp[:])

            def stage_h(t):
                xT = xT_tiles.pop(t)
                # fp8 view: [p, i, (j n)] where i selects the byte (d-half)
                rhs = xT[:].bitcast(F8).rearrange("p j (n i) -> p i (j n)", i=2)
                ph = php.tile([H, CHUNK], F32)
                for u in range(CHUNK // 512):
                    rv = rhs[:, :, u * 512 : (u + 1) * 512]
                    nc.tensor.matmul(
                        ph[:, u * 512 : (u + 1) * 512], w1hi, rv,
                        start=True, stop=False, perf_mode=DR,
                    )
                    nc.tensor.matmul(
                        ph[:, u * 512 : (u + 1) * 512], w1lo, rv,
                        start=False, stop=True, perf_mode=DR,
                    )
                th = thp.tile([H, CHUNK], BF16)
                th_tiles[t] = th
                nc.scalar.activation(
                    th[:], ph[:], TANH, bias=b1, scale=1.0 / (XS * WS)
                )

            def stage_s(t):
                th = th_tiles.pop(t)
                if t % 2 == 0:
                    ps_tiles[t] = psp.tile([128, 2, TPC], F32, name="ps", tag="ps")
                ps = ps_tiles[t - t % 2]
                for j in range(TPC):
                    nc.tensor.matmul(
                        ps[:, t % 2, j : j + 1],
                        th[:, j * 128 : (j + 1) * 128],
                        w2,
                        start=True,
                        stop=True,
                    )

            def stage_exp(a):
                # pair (a, a+1); a even. Covers one or two chunks.
                w = min(2, nchunks - a)
                ps = ps_tiles.pop(a)
                eb = ebp.tile([128, 2, TPC], BF16)
                eb_tiles[a] = eb
                nc.scalar.activation(
                    eb[:, 0:w], ps[:, 0:w], EXP, bias=0.0, scale=1.0
                )

            def stage_sw(t):
                eb = eb_tiles[t - t % 2]
                if t % 2 == 1 or t == nchunks - 1:
                    del eb_tiles[t - t % 2]
                tm = swp.tile([128, TPC, S], BF16, tag="tm")
                nc.vector.tensor_tensor(
                    tm[:],
                    delta[:, t * TPC : (t + 1) * TPC].unsqueeze(2).broadcast_to(
                        [128, TPC, S]
                    ),
                    kvec.unsqueeze(1).broadcast_to([128, TPC, S]),
                    AluOpType.is_equal,
                )
                sw = swp.tile([128, TPC, S], BF16, tag="sw")
                nc.vector.tensor_tensor(
                    sw[:],
                    tm[:],
                    eb[:, t % 2].unsqueeze(2).broadcast_to([128, TPC, S]),
                    AluOpType.mult,
                )
                return sw

            def stage_o(t, sw):
                g, fi = divmod(t, FLUSH)
                if fi == 0:
                    pP_tiles[g] = pPp.tile(
                        [128, FLUSH, 3, TPC, S], F32, name="pP", tag="pP"
                    )
                pP = pP_tiles[g]
                xn8 = slots[t % NSLOT][:].bitcast(F8)  # [128, TPC, 256]
                for j in range(TPC):
                    nc.tensor.matmul(
                        pP[:, fi, 0, j, :], xn8[:, j, 0:128], sw[:, j, :],
                        start=True, stop=True,
                    )
                    nc.tensor.matmul(
                        pP[:, fi, 1, j, :], xn8[:, j, 128:256], sw[:, j, :],
                        start=True, stop=True,
                    )
                    nc.tensor.matmul(
                        pP[0:1, fi, 2, j, :], ones8, sw[:, j, :],
                        start=True, stop=True,
                    )
                # flush the group once its last chunk is done
                if fi == FLUSH - 1 or t == nchunks - 1:
                    pP = pP_tiles.pop(g)
                    nw = fi + 1
                    nc.vector.tensor_copy(
                        psb[:, g * FLUSH * pcols : (g * FLUSH + nw) * pcols]
                        .rearrange("p (f c) -> p f c", f=nw),
                        pP[:, 0:nw].rearrange("p f h j k -> p f (h j k)"),
                    )
                    nc.sync.dma_start(
                        o_d[:, g * FLUSH * pcols : (g * FLUSH + nw) * pcols],
                        psb[:, g * FLUSH * pcols : (g * FLUSH + nw) * pcols],
                    )

            LAG_H, LAG_S, LAG_E, LAG_W, LAG_O = 2, 4, 6, 7, 8
            for t in range(nchunks + LAG_O):
                if t + 3 < nchunks:
                    nat_dma(t + 3)
                a = t - LAG_E
                if 0 <= a < nchunks and a % 2 == 0:
                    stage_exp(a)
                if t < nchunks:
                    stage_tp(t)
                if 0 <= t - LAG_H < nchunks:
                    stage_h(t - LAG_H)
                if 0 <= t - LAG_S < nchunks:
                    stage_s(t - LAG_S)
                if 0 <= t - LAG_W < nchunks:
                    sw_tiles[t - LAG_W] = stage_sw(t - LAG_W)
                if 0 <= t - LAG_O < nchunks:
                    stage_o(t - LAG_O, sw_tiles.pop(t - LAG_O))

    nc.compile()
    return nc


def kernel(x, batch, W1, b1, W2, b2):
    x = np.asarray(x, np.float32)
    batch = np.asarray(batch)
    W1 = np.asarray(W1, np.float32)
    b1 = np.asarray(b1, np.float32)
    W2 = np.asarray(W2, np.float32)

    bat = batch.astype(np.int64)
    N = bat.shape[0]
    bounds = np.searchsorted(bat, np.arange(0, G + 1, SPC), side="left")
    ncounts = np.diff(bounds)
    npad = int(-(-ncounts.max() // CHUNK) * CHUNK)
    ntiles = npad // 128
    nchunks = npad // CHUNK

    counts = np.bincount(bat, minlength=G).astype(np.float32)

    # global fp8 quantization (scaled), plus exact residual for the host-side
    # correction term
    x8 = (x * XS).astype(NPF8)
    x8f = x8.astype(np.float32)
    resid = x - x8f * (1.0 / XS)  # exact in f32

    # per-segment sums of the residual -> correction (sum x - sum x8)/c^2
    seg_starts = np.searchsorted(bat, np.arange(G), side="left")
    rsum = np.add.reduceat(resid, np.minimum(seg_starts, N - 1), axis=0)
    # reduceat yields a[i] (not 0) for empty segments; zero those out
    rsum[counts == 0] = 0.0

    cg = np.maximum(counts, 1.0)
    ccorr = rsum / (cg * cg)[:, None]  # [G, D]

    # W1 two-term fp8 split (scaled by WS); DoubleRow pack [c, i, h]
    w1s = W1 * WS
    w1hi8 = w1s.astype(NPF8)
    w1lo8 = (w1s - w1hi8.astype(np.float32)).astype(NPF8)
    w1hi = np.ascontiguousarray(
        np.stack([w1hi8[:128], w1hi8[128:]], axis=1)
    )  # [128, 2, H] fp8
    w1lo = np.ascontiguousarray(np.stack([w1lo8[:128], w1lo8[128:]], axis=1))

    # per-core prep
    in_maps = []
    metas = []
    S = 2
    core_data = []
    for c in range(C):
        s, e = bounds[c], bounds[c + 1]
        nct = e - s
        locseg = (bat[s:e] - c * SPC).astype(np.int64)
        g0 = np.zeros(ntiles, np.int64)
        nvalid_tiles = -(-nct // 128)
        if nct:
            g0[:nvalid_tiles] = locseg[np.arange(nvalid_tiles) * 128]
        dlt = np.full(npad, -1.0, np.float32)
        if nct:
            dlt[:nct] = locseg - g0[np.arange(nct) // 128]
        smax = int(dlt.max()) + 1 if nct else 1
        core_data.append((s, e, nct, g0, dlt))
        S = max(S, smax)

    key = (npad, S)
    if key not in _cache_s:
        _cache_s[key] = _build(npad, S)
    nc = _cache_s[key]
    _cache[npad] = nc  # test.py compatibility

    pcols = 3 * TPC * S
    boff, CB = _blob_layout(ntiles, S)

    for c in range(C):
        s, e, nct, g0, dlt = core_data[c]
        xpad = np.zeros((npad, D), NPF8)
        xpad[:nct] = x8[s:e]
        xb = xpad.view(np.uint8)
        pk = (
            xb[:, :128].astype(np.uint16)
            | (xb[:, 128:].astype(np.uint16) << 8)
        )  # [npad, 128] uint16
        xp = np.ascontiguousarray(
            pk.reshape(ntiles, 128, 128).transpose(1, 0, 2).reshape(128, ntiles * 128)
        ).view(NPBF)

        # constant blob, byte-assembled then viewed as bf16 columns
        bb = np.zeros((128, CB * 2), np.uint8)

        def put(name, arr_bytes):
            a, b = boff[name]
            bb[:, a * 2 : a * 2 + arr_bytes.shape[1]] = arr_bytes

        put("w1hi", w1hi.reshape(128, 256).view(np.uint8))
        put("w1lo", w1lo.reshape(128, 256).view(np.uint8))
        put("ident", np.eye(128, dtype=NPBF).view(np.uint8))
        dl = np.ascontiguousarray(dlt.reshape(ntiles, 128).T.astype(NPBF))
        put("delta", dl.view(np.uint8))
        put("b1", b1.reshape(H, 1).astype(np.float32).view(np.uint8))
        put("w2", W2.reshape(H, 1).astype(NPBF).view(np.uint8))
        kv = np.broadcast_to(
            np.arange(S, dtype=np.float32)[None, :], (128, S)
        ).astype(NPBF)
        put("kvec", np.ascontiguousarray(kv).view(np.uint8))
        put("ones8", np.ones((128, 1), NPF8).view(np.uint8))

        in_maps.append({"xp": xp, "cblob": bb.view(NPBF)})
        metas.append((g0, nct))

    res = bass_utils.run_bass_kernel_spmd(nc, in_maps, core_ids=list(range(C)))

    # host-side finish: window->segment reduction, normalize, unpermute, correct
    f_idx = np.arange(256)
    d_of_f = f_idx // 2 + 128 * (f_idx % 2)
    f_of_d = np.empty(256, np.int64)
    f_of_d[d_of_f] = f_idx

    out = np.zeros((G, D), np.float32)
    for c in range(C):
        g0, nct = metas[c]
        o = res.results[c]["o"].reshape(128, nchunks, 3, TPC, S)
        # P rows: [q, t, half, j, k]; f = half*128 + q
        P = np.concatenate([o[:, :, 0], o[:, :, 1]], axis=0)  # [256, t, j, k]
        De = o[0, :, 2]  # [t, j, k]
        P = P.reshape(256, ntiles, S)
        De = De.reshape(ntiles, S)
        wseg = np.minimum(g0[:, None] + np.arange(S)[None, :], SPC - 1)
        U = np.zeros((SPC, 256), np.float64)
        np.add.at(U, wseg.ravel(), P.reshape(256, -1).T.astype(np.float64))
        DeU = np.zeros(SPC, np.float64)
        np.add.at(DeU, wseg.ravel(), De.ravel().astype(np.float64))
        cgl = cg[c * SPC : (c + 1) * SPC]
        y = U[:, f_of_d] / (XS * np.maximum(DeU, 1e-30) * cgl)[:, None]
        out[c * SPC : (c + 1) * SPC] = y.astype(np.float32) + ccorr[
            c * SPC : (c + 1) * SPC
        ]
    return out
